# revision 43
# baseline (speedup 1.0000x reference)
"""GPT-2-ish forward (B=4, T=1024, D=768, H=12, L=2, V=50257) on 8 trn2 cores.

Sharding: core pair (2b, 2b+1) sequence-parallel over batch b's tokens:
parity p owns interleaved 128-token chunks {p, p+2, p+4, p+6} (512 tokens).
Per layer each core LNs its own tokens, AllGathers h within the pair (two
256-token chunks, pipelined against QKV compute), computes K/V for all 1024
tokens and Q/attention/proj/MLP for its own 512. lm_head: own tokens x full
vocab (padded to 50688), so no final exchange is needed.

On-device layout: activations [features, tokens]; residual fp32 resident.
Attention: scores per (head, key-tile) with kT stationary; exp on Scalar
(unnormalized, causal mask multiplied after); att@V with V stationary
[128, 65] (ones column appended so the softmax denominator lands in psum
partition 64); normalization = reciprocal + gpsimd partition_broadcast +
vector multiply, emitting attoT directly in [hd, tokens] layout (no
transposes). LayerNorm: column sums via ones-vector matmuls, mean/rstd
broadcasts materialized as rank-1 matmuls in PSUM, apply fused as
sub / mult / two-scalar tensor_scalar on Vector. All matmuls bf16 with
fp32 PSUM; logits evicted f16 and upcast on host.
"""

import numpy as np
import ml_dtypes
from contextlib import ExitStack

import concourse.bass as bass
from concourse import bacc
import concourse.mybir as mybir
import concourse.tile as tile
from concourse.bass_utils import run_bass_kernel_spmd

BF16 = mybir.dt.bfloat16
F32 = mybir.dt.float32
F16 = mybir.dt.float16
AF = mybir.ActivationFunctionType
ALU = mybir.AluOpType

V = 50257
VPAD = 50688          # 99 * 512
D = 768
H = 12
HD = 64
L = 2
T = 1024
B = 4
TOWN = 512            # tokens owned per core
EPS = 1e-5
NKT = D // 128        # 6 feature tiles
NTT = T // 128        # 8 global token tiles
NOS = TOWN // 128     # 4 own token sub-chunks
NVC = VPAD // 512     # 99 lm vocab chunks
PAIRS = [[0, 1], [2, 3], [4, 5], [6, 7]]

TRACE = False
LAST_RESULT = None

_S = {}


def _ln_phase(tc, nc, tag, xt, g_col, b_col, hout, small, scratch, lnps,
              post_half=None):
    """LayerNorm over features (partition dim) of own tokens, processed in
    two 256-token column halves so consumers (the AllGather) start early.
    xt: [128, NKT, TOWN] f32; g_col/b_col: [128, NKT] f32 per-feature params.
    hout: list of two bf16 APs [128, NKT, 256], one per column half.
    post_half(ch) is called after half ch."""
    ones_bf = _S["ones_bf"]
    ones_row = _S["ones_row"]
    eps_sb = _S["eps_sb"]

    for ch in range(2):
        cs = slice(ch * 256, (ch + 1) * 256)
        # full-bank padding: s1/s2 accumulation groups interleave, and a
        # start=True clears has_written bank-wide — they must not share one
        s1 = lnps.tile([1, 256], F32, tag="s1", name="s1",
                       padded_shape=[128, 512])
        s2 = lnps.tile([1, 256], F32, tag="s2", name="s2",
                       padded_shape=[128, 512])
        for kt in range(NKT):
            xbf = scratch.tile([128, 256], BF16, tag="xbf", name="xbf")
            sq = scratch.tile([128, 256], BF16, tag="sq", name="sq")
            xs = xt[:, kt, cs]
            nc.vector.tensor_copy(xbf, xs)
            nc.vector.tensor_mul(sq, xs, xs)
            nc.tensor.matmul(s1, ones_bf, xbf, start=(kt == 0),
                             stop=(kt == NKT - 1))
            nc.tensor.matmul(s2, ones_bf, sq, start=(kt == 0),
                             stop=(kt == NKT - 1))
        # mean = s1/D ; var = s2/D - mean^2 ; rstd = 1/sqrt(var+eps)
        mean = small.tile([1, 256], F32, tag="mean", name="mean")
        var = small.tile([1, 256], F32, tag="var", name="var")
        rstd = small.tile([1, 256], F32, tag="rstd", name="rstd")
        mean_bf = small.tile([1, 256], BF16, tag="mean_bf", name="mean_bf")
        rstd_bf = small.tile([1, 256], BF16, tag="rstd_bf", name="rstd_bf")
        nc.vector.tensor_scalar_mul(mean, s1, 1.0 / D)
        nc.vector.tensor_mul(var, mean, mean)
        nc.vector.scalar_tensor_tensor(var, s2, 1.0 / D, var,
                                       op0=ALU.mult, op1=ALU.subtract)
        nc.scalar.activation(var, var, AF.Sqrt, bias=eps_sb)
        nc.vector.reciprocal(rstd, var)
        nc.vector.tensor_copy(mean_bf, mean)
        nc.vector.tensor_copy(rstd_bf, rstd)
        # broadcast fields mb = 1 (x) mean, rb = 1 (x) rstd  [128, 256] psum
        mb = lnps.tile([128, 256], F32, tag="mb", name="mb")
        rb = lnps.tile([128, 256], F32, tag="rb", name="rb")
        nc.tensor.matmul(mb, ones_row[0:1, 0:128], mean_bf, start=True,
                         stop=True)
        nc.tensor.matmul(rb, ones_row[0:1, 0:128], rstd_bf, start=True,
                         stop=True)
        # apply: h = ((x - mb) * rb) * g + b
        for kt in range(NKT):
            tmp = scratch.tile([128, 256], F32, tag="lntmp", name="lntmp")
            nc.vector.tensor_sub(tmp, xt[:, kt, cs], mb)
            nc.vector.tensor_mul(tmp, tmp, rb)
            nc.vector.tensor_scalar(hout[ch][:, kt, :], tmp,
                                    g_col[:, kt:kt + 1], b_col[:, kt:kt + 1],
                                    op0=ALU.mult, op1=ALU.add)
        if post_half is not None:
            post_half(ch)


def build_bass():
    nc = bacc.Bacc(None, target_bir_lowering=False)
    # ---- DRAM I/O (per-core) ----
    xT_d = nc.dram_tensor("xT", [D, TOWN], F32, kind="ExternalInput")
    h0_d = nc.dram_tensor("h0", [D, T], BF16, kind="ExternalInput")
    h0own_d = nc.dram_tensor("h0own", [D, TOWN], BF16, kind="ExternalInput")
    qkw_d = nc.dram_tensor("qkw", [L, D, 2 * D], BF16, kind="ExternalInput")
    vw_d = nc.dram_tensor("vw", [L, D, D], BF16, kind="ExternalInput")
    pw_d = nc.dram_tensor("pw", [L, D, D], BF16, kind="ExternalInput")
    fcw_d = nc.dram_tensor("fcw", [L, D, 4 * D], BF16, kind="ExternalInput")
    fc2w_d = nc.dram_tensor("fc2w", [L, 4 * D, D], BF16, kind="ExternalInput")
    qkb_d = nc.dram_tensor("qkb", [L, 2 * D], F32, kind="ExternalInput")
    vb_d = nc.dram_tensor("vb", [L, D], BF16, kind="ExternalInput")
    pb_d = nc.dram_tensor("pb", [L, D], F32, kind="ExternalInput")
    fcb_d = nc.dram_tensor("fcb", [L, 4 * D], F32, kind="ExternalInput")
    fc2b_d = nc.dram_tensor("fc2b", [L, D], F32, kind="ExternalInput")
    ln_d = nc.dram_tensor("lnp", [L, 4, D], F32, kind="ExternalInput")  # g1,b1,g2,b2
    lnf_d = nc.dram_tensor("lnf", [2, D], F32, kind="ExternalInput")
    mask_d = nc.dram_tensor("mask", [128, 256], BF16, kind="ExternalInput")
    lmw_d = nc.dram_tensor("lmw", [D, VPAD], BF16, kind="ExternalInput")
    out_d = nc.dram_tensor("out", [TOWN, VPAD], F16, kind="ExternalOutput")

    with tile.TileContext(nc) as tc, ExitStack() as octx:
        singles = octx.enter_context(tc.tile_pool(name="singles", bufs=1))
        resid = octx.enter_context(tc.tile_pool(name="resid", bufs=1))
        dram = octx.enter_context(tc.tile_pool(name="dram", bufs=2, space="DRAM"))

        ones_bf = singles.tile([128, 1], BF16)
        nc.vector.memset(ones_bf, 1.0)
        ones_row = singles.tile([1, 512], BF16)
        nc.vector.memset(ones_row, 1.0)
        eps_sb = singles.tile([1, 1], F32)
        nc.vector.memset(eps_sb, EPS)
        _S["ones_bf"] = ones_bf
        _S["ones_row"] = ones_row
        _S["eps_sb"] = eps_sb

        # mask for the last two key tiles of any strip s: cols 0:128 = tile
        # kt=2s (p=0: lower-tri, p=1: ones), cols 128:256 = tile kt=2s+1
        # (p=0: zeros, p=1: lower-tri). DMA deferred below so layer-0's
        # critical h0own transfer goes out first.
        mask_sb = singles.tile([128, 256], BF16)

        # residual stream (own tokens), fp32, resident (DMA deferred)
        xt = resid.tile([128, NKT, TOWN], F32)

        # warm-up collective: absorbs the first-call ncfw latency during
        # layer-0 compute so layer 1's real AllGather is not delayed
        wuin = dram.tile([128, 16], BF16, tag="wuin", name="wuin")
        wuout = dram.tile([2, 128, 16], BF16, tag="wuout", name="wuout")
        nc.gpsimd.dma_start(wuin[:, :], mask_d[:, 0:16])
        nc.gpsimd.collective_compute(
            "AllGather", ALU.bypass, replica_groups=PAIRS,
            ins=[wuin.opt()], outs=[wuout.opt()])

        for l in range(L):
            with ExitStack() as lctx:
                lnpool = lctx.enter_context(tc.tile_pool(name=f"ln{l}", bufs=1))
                wpool = lctx.enter_context(tc.tile_pool(name=f"w{l}", bufs=3))
                biasp = lctx.enter_context(tc.tile_pool(name=f"bias{l}", bufs=1))
                small = lctx.enter_context(tc.tile_pool(name=f"small{l}", bufs=2))
                scratch = lctx.enter_context(tc.tile_pool(name=f"scr{l}", bufs=3))

                qkb_sb = biasp.tile([128, 12], F32)
                nc.sync.dma_start(qkb_sb, qkb_d[l].rearrange("(t p) -> p t", p=128))
                vbbf_sb = biasp.tile([1, D], BF16)
                nc.sync.dma_start(vbbf_sb, vb_d[l].rearrange("(o d) -> o d", o=1))
                pb_sb = biasp.tile([128, 6], F32)
                nc.sync.dma_start(pb_sb, pb_d[l].rearrange("(t p) -> p t", p=128))
                fcb_sb = biasp.tile([128, 24], F32)
                nc.sync.dma_start(fcb_sb, fcb_d[l].rearrange("(t p) -> p t", p=128))
                fc2b_sb = biasp.tile([128, 6], F32)
                nc.sync.dma_start(fc2b_sb, fc2b_d[l].rearrange("(t p) -> p t", p=128))
                ln_sb = biasp.tile([128, 4, NKT], F32)
                nc.sync.dma_start(ln_sb, ln_d[l].rearrange("g (k p) -> p g k", p=128))

                # ---------- LN1 (own tokens) + AllGather h within pair ----
                # hfull split per gathered chunk so chunk-0 consumers do not
                # wait on the chunk-1 collective (dep tracking is per-tile).
                h = lnpool.tile([128, NKT, TOWN], BF16, tag="h", name="h")
                hfull = [lnpool.tile([128, NKT, 512], BF16, tag=f"hfull{c}",
                                     name=f"hfull{c}") for c in range(2)]

                def kick_ag(ch):
                    cs = slice(ch * 256, (ch + 1) * 256)
                    agin = dram.tile([D, 256], BF16, tag="agin", name="agin")
                    agout = dram.tile([2, D, 256], BF16, tag="agout", name="agout")
                    nc.sync.dma_start(
                        agin.rearrange("(k p) t -> p k t", p=128), h[:, :, cs])
                    nc.gpsimd.collective_compute(
                        "AllGather", ALU.bypass, replica_groups=PAIRS,
                        ins=[agin.opt()], outs=[agout.opt()])
                    # global chunk 4ch + 2s + r lands at block position s*256+r*128
                    hdst = hfull[ch].rearrange("p k (s r t) -> r p k s t",
                                               s=2, r=2, t=128)
                    for r in range(2):
                        asrc = agout[r].rearrange("(k p) (s t) -> s p k t",
                                                  p=128, t=128)
                        for s in range(2):
                            nc.sync.dma_start(hdst[r, :, :, s, :], asrc[s])

                if l == 0:
                    # layer 0: LN1(x0) is precomputed on host — no collective.
                    # h0own first: it gates qT, the kernel's first matmuls.
                    nc.sync.dma_start(
                        h, h0own_d.rearrange("(k p) t -> p k t", p=128))
                    for c in range(2):
                        nc.sync.dma_start(
                            hfull[c], h0_d[:, c * 512:(c + 1) * 512]
                            .rearrange("(k p) t -> p k t", p=128))
                    nc.sync.dma_start(mask_sb, mask_d[:, :])
                    nc.sync.dma_start(
                        xt, xT_d.rearrange("(k p) t -> p k t", p=128))
                else:
                    with tc.tile_pool(name=f"lnps{l}a", bufs=1,
                                      space="PSUM") as lnps:
                        _ln_phase(tc, nc, f"l{l}a", xt, ln_sb[:, 0, :],
                                  ln_sb[:, 1, :],
                                  [h[:, :, 0:256], h[:, :, 256:512]],
                                  small, scratch, lnps, post_half=kick_ag)

                # ---------- qT own, kT/V full, early scores --------------
                # Attention strips: per (pr, hh, s) the scores for own
                # q-sub-chunk s (128 cols) cover key tiles kt=0..2s+1 and are
                # packed [128, (2s+2)*128] in psum; exp evicts to bf16 strips.
                # s=0,1 (kt<=3, needs only AG chunk 0) run for all pr between
                # kT chunk 0 and kT chunk 1 to cover the AllGather latency.
                q_sb = lnpool.tile([128, NKT, TOWN], BF16, tag="q_sb", name="q_sb")
                k_sb = lnpool.tile([128, NKT, T], BF16, tag="k_sb", name="k_sb")
                attT01 = lnpool.tile([128, NKT, 2, 768], BF16, tag="attT01",
                                     name="attT01")
                v_aug = [lnpool.tile([128, 12, 65], BF16, tag=f"vaug{i}",
                                     name=f"vaug{i}") for i in range(NTT)]
                fcw_sb = wpool.tile([128, NKT, 4 * D], BF16, tag="fcw_l",
                                    name="fcw_sb", bufs=1)

                def scores_strip(sps_pool, stag, twid, pr, hh, s, dst, sbufs=3):
                    """Scores+exp+mask for strip (pr, hh, s) -> dst bf16 AP."""
                    nk = 2 * s + 2
                    hs = slice(hh * 64, hh * 64 + 64)
                    st = sps_pool.tile([128, twid], F32, tag=stag, name=stag,
                                       bufs=sbufs)
                    for kt in range(nk):
                        nc.tensor.matmul(
                            st[:, kt * 128:(kt + 1) * 128],
                            k_sb[hs, pr, kt * 128:(kt + 1) * 128],
                            q_sb[hs, pr, s * 128:(s + 1) * 128],
                            start=True, stop=True)
                    for a0 in range(0, nk * 128, 512):
                        a1 = min(nk * 128, a0 + 512)
                        nc.scalar.activation(dst[:, a0:a1], st[:, a0:a1],
                                             AF.Exp, scale=0.125)
                    # only the last two key tiles (kt=2s, 2s+1) need masking
                    nc.vector.tensor_mul(dst[:, (nk - 2) * 128:nk * 128],
                                         dst[:, (nk - 2) * 128:nk * 128],
                                         mask_sb)

                def kv_chunk(qkps, ch):
                    gs = slice(ch * 512, (ch + 1) * 512)
                    for f in range(NKT):
                        wt = wpool.tile([128, NKT, 128], BF16, tag="kw_t",
                                        name="kw_t", bufs=2)
                        nc.sync.dma_start(
                            wt, qkw_d[l][:, D + f * 128:D + (f + 1) * 128]
                            .rearrange("(t p) f -> p t f", p=128))
                        ps = qkps.tile([128, 512], F32, tag="qkps", name="qkps",
                                       bufs=2)
                        for kt in range(NKT):
                            nc.tensor.matmul(ps, wt[:, kt, :], hfull[ch][:, kt, :],
                                             start=(kt == 0),
                                             stop=(kt == NKT - 1))
                        nc.vector.tensor_scalar_add(k_sb[:, f, gs], ps,
                                                    qkb_sb[:, 6 + f:7 + f])
                    for tt in range(ch * 4, ch * 4 + 4):
                        nc.vector.memset(v_aug[tt][:, :, 64:65], 1.0)
                        for vc in range(2):
                            vs = slice(vc * 384, (vc + 1) * 384)
                            ps = qkps.tile([128, 384], F32, tag="vps", name="vps",
                                           bufs=2)
                            for kt in range(NKT):
                                nc.tensor.matmul(
                                    ps, hfull[ch][:, kt,
                                                  (tt % 4) * 128:(tt % 4 + 1) * 128],
                                    vw_sb[kt][:, vs],
                                    start=(kt == 0), stop=False)
                            nc.tensor.matmul(ps, ones_row[:, 0:128],
                                             vbbf_sb[:, vs],
                                             start=False, stop=True)
                            nc.vector.tensor_copy(
                                v_aug[tt][:, vc * 6:(vc + 1) * 6, 0:64],
                                ps.rearrange("p (h d) -> p h d", d=64))

                with tc.tile_pool(name=f"qkps{l}", bufs=3, space="PSUM") as qkps:
                    # qT in token halves so it starts right after LN1 half 0
                    # (keeps the PE warm through the LN1 tail / AG wait)
                    qw_sb = wpool.tile([128, NKT, D], BF16, tag="qw_sb",
                                       name="qw_sb", bufs=1)
                    nc.sync.dma_start(
                        qw_sb, qkw_d[l][:, 0:D]
                        .rearrange("(t p) f -> p t f", p=128))
                    for ch in range(2):
                        cs = slice(ch * 256, (ch + 1) * 256)
                        for f in range(NKT):
                            ps = qkps.tile([128, 256], F32, tag="qps",
                                           name="qps", bufs=2)
                            for kt in range(NKT):
                                nc.tensor.matmul(
                                    ps, qw_sb[:, kt, f * 128:(f + 1) * 128],
                                    h[:, kt, cs],
                                    start=(kt == 0), stop=(kt == NKT - 1))
                            nc.vector.tensor_scalar_add(q_sb[:, f, cs], ps,
                                                        qkb_sb[:, f:f + 1])

                    vw_sb = [wpool.tile([128, D], BF16, tag=f"vw{i}",
                                        name=f"vw{i}", bufs=1) for i in range(NKT)]
                    for kt in range(NKT):
                        nc.sync.dma_start(vw_sb[kt],
                                          vw_d[l][kt * 128:(kt + 1) * 128, :])
                    kv_chunk(qkps, 0)
                    # early scores s=0,1 for all pr (covers AG chunk 1 wait)
                    nc.sync.dma_start(fcw_sb, fcw_d[l]
                                      .rearrange("(t p) f -> p t f", p=128))
                    for pr in range(NKT):
                        for hh in range(2):
                            for s in range(2):
                                scores_strip(
                                    qkps, "stripA", TOWN, pr, hh, s,
                                    attT01[:, pr, hh,
                                           s * 256:s * 256 + (2 * s + 2) * 128],
                                    sbufs=2)
                    kv_chunk(qkps, 1)

                # ---------- attention tail per head-pair ------------------
                attoT = lnpool.tile([128, NKT, TOWN], BF16, tag="attoT",
                                    name="attoT")
                with tc.tile_pool(name=f"sps{l}", bufs=3, space="PSUM") as sps, \
                     tc.tile_pool(name=f"ops{l}", bufs=2, space="PSUM") as ops, \
                     tc.tile_pool(name=f"attp{l}", bufs=3) as attp:
                    attT23s = {}

                    def attv(pr, hh):
                        hcol = 2 * pr + hh
                        attT23 = attT23s[pr]

                        def att_src(s, kt):
                            if s < 2:
                                base = s * 256
                                return attT01[:, pr, hh,
                                              base + kt * 128:
                                              base + (kt + 1) * 128]
                            base = (s - 2) * 768
                            return attT23[:, hh, base + kt * 128:
                                          base + (kt + 1) * 128]

                        po = ops.tile([65, TOWN], F32, tag=f"po{hh}",
                                      name=f"po{hh}", bufs=1)
                        # start=True clears has_written for the whole bank,
                        # so only the first MM opens the group; later
                        # regions overwrite-on-first-touch.
                        for kt in range(NTT):
                            for s in range(kt // 2, 4):
                                nc.tensor.matmul(
                                    po[:, s * 128:(s + 1) * 128],
                                    v_aug[kt][:, hcol, :], att_src(s, kt),
                                    start=(kt == 0 and s == 0),
                                    stop=(kt == 2 * s + 1),
                                    skip_group_check=True)
                        r_sb = scratch.tile([1, TOWN], F32, tag="r_sb",
                                            name="r_sb", bufs=2)
                        rbc = scratch.tile([64, TOWN], F32, tag="rbc",
                                           name="rbc", bufs=2)
                        nc.vector.reciprocal(r_sb, po[64:65, :])
                        nc.gpsimd.partition_broadcast(rbc, r_sb, channels=64)
                        nc.vector.tensor_mul(
                            attoT[hh * 64:hh * 64 + 64, pr, :],
                            po[0:64, :], rbc)

                    # software pipeline: chain i's scores overlap chain i-2's
                    # att@V (two chains of slack so attv never waits on exp).
                    chains = [(pr, hh) for pr in range(NKT) for hh in range(2)]
                    for i, (pr, hh) in enumerate(chains):
                        if hh == 0:
                            attT23s[pr] = attp.tile([128, 2, 1792], BF16,
                                                    tag="attT23", name="attT23")
                        for s in range(2, 4):
                            scores_strip(
                                sps, "stripB", 1024, pr, hh, s,
                                attT23s[pr][:, hh, (s - 2) * 768:
                                            (s - 2) * 768 + (2 * s + 2) * 128])
                        if i >= 2:
                            attv(*chains[i - 2])
                    attv(*chains[-2])
                    attv(*chains[-1])

                # ---------- proj + residual ----------
                with tc.tile_pool(name=f"pps{l}", bufs=3, space="PSUM") as pps:
                    for ot in range(NKT):
                        wt = wpool.tile([128, NKT, 128], BF16, tag="pw_t",
                                        name="pw_t")
                        nc.sync.dma_start(
                            wt, pw_d[l][:, ot * 128:(ot + 1) * 128]
                            .rearrange("(t p) f -> p t f", p=128))
                        ps = pps.tile([128, TOWN], F32, tag="pps", name="pps")
                        for kt in range(NKT):
                            nc.tensor.matmul(ps, wt[:, kt, :], attoT[:, kt, :],
                                             start=(kt == 0), stop=(kt == NKT - 1))
                        nc.vector.scalar_tensor_tensor(
                            xt[:, ot, :], ps, pb_sb[:, ot:ot + 1],
                            xt[:, ot, :], op0=ALU.add, op1=ALU.add)

                # ---------- LN2 + MLP (own tokens) ----------
                h2 = lnpool.tile([128, NKT, TOWN], BF16, tag="h", name="h2")
                with tc.tile_pool(name=f"lnps{l}b", bufs=1, space="PSUM") as lnps:
                    _ln_phase(tc, nc, f"l{l}b", xt, ln_sb[:, 2, :], ln_sb[:, 3, :],
                              [h2[:, :, 0:256], h2[:, :, 256:512]],
                              small, scratch, lnps)

                with tc.tile_pool(name=f"mlpps{l}", bufs=3, space="PSUM") as mlpps, \
                     tc.tile_pool(name=f"h2p{l}", bufs=1) as h2p:
                    h2c = h2p.tile([128, 24, TOWN], BF16, tag="h2c", name="h2c")
                    # fc in token halves: half 0 starts right after LN2 half 0
                    for ch in range(2):
                        cs = slice(ch * 256, (ch + 1) * 256)
                        for f in range(24):
                            ps = mlpps.tile([128, 256], F32, tag="fcps",
                                            name="fcps")
                            for kt in range(NKT):
                                nc.tensor.matmul(
                                    ps, fcw_sb[:, kt, f * 128:(f + 1) * 128],
                                    h2[:, kt, cs],
                                    start=(kt == 0), stop=(kt == NKT - 1))
                            nc.scalar.activation(h2c[:, f, cs], ps,
                                                 AF.Gelu_apprx_tanh,
                                                 bias=fcb_sb[:, f:f + 1])
                    for ot in range(NKT):
                        wt = wpool.tile([128, 24, 128], BF16, tag="fc2w_t",
                                        name="fc2w_t", bufs=2)
                        nc.sync.dma_start(
                            wt, fc2w_d[l][:, ot * 128:(ot + 1) * 128]
                            .rearrange("(t p) f -> p t f", p=128))
                        ps = mlpps.tile([128, TOWN], F32, tag="fc2ps", name="fc2ps")
                        for kt in range(24):
                            nc.tensor.matmul(ps, wt[:, kt, :], h2c[:, kt, :],
                                             start=(kt == 0), stop=(kt == 23))
                        nc.vector.scalar_tensor_tensor(
                            xt[:, ot, :], ps, fc2b_sb[:, ot:ot + 1],
                            xt[:, ot, :], op0=ALU.add, op1=ALU.add)

        # ---------- final LN + lm_head (own tokens, full vocab) ----------
        with ExitStack() as fctx:
            lnpool = fctx.enter_context(tc.tile_pool(name="lnfp", bufs=1))
            biasp = fctx.enter_context(tc.tile_pool(name="biasf", bufs=1))
            small = fctx.enter_context(tc.tile_pool(name="smallf", bufs=2))
            scratch = fctx.enter_context(tc.tile_pool(name="scrf", bufs=3))
            lnf_sb = biasp.tile([128, 2, NKT], F32)
            nc.sync.dma_start(lnf_sb, lnf_d.rearrange("g (k p) -> p g k", p=128))
            # xf in two half-tiles so lm token tiles 0/1 start after LNf half 0
            xf01 = lnpool.tile([128, NKT, 256], BF16, tag="xf01", name="xf01")
            xf23 = lnpool.tile([128, NKT, 256], BF16, tag="xf23", name="xf23")
            with tc.tile_pool(name="lnpsf", bufs=1, space="PSUM") as lnps:
                _ln_phase(tc, nc, "lf", xt, lnf_sb[:, 0, :], lnf_sb[:, 1, :],
                          [xf01, xf23], small, scratch, lnps)

            # vocab blocks of 4 share each stationary (xf) load; vector-only
            # eviction keeps Scalar out of the lm pipeline.
            with tc.tile_pool(name="lmw", bufs=2) as lmwp, \
                 tc.tile_pool(name="lmps", bufs=2, space="PSUM") as lmps, \
                 tc.tile_pool(name="lmev", bufs=6) as lmev:
                for vb in range(0, NVC, 4):
                    vcs = list(range(vb, min(vb + 4, NVC)))
                    wts = []
                    for j, vc in enumerate(vcs):
                        wt = lmwp.tile([128, NKT, 512], BF16, tag=f"lmw{j}",
                                       name=f"lmw{j}")
                        nc.sync.dma_start(
                            wt, lmw_d[:, vc * 512:(vc + 1) * 512]
                            .rearrange("(t p) v -> p t v", p=128))
                        wts.append(wt)
                    for tt in range(NOS):
                        pss = [lmps.tile([128, 512], F32, tag=f"lmps{j}",
                                         name=f"lmps{j}")
                               for j in range(len(vcs))]
                        xfh = xf01 if tt < 2 else xf23
                        tc0 = (tt % 2) * 128
                        for kt in range(NKT):
                            for j in range(len(vcs)):
                                nc.tensor.matmul(
                                    pss[j], xfh[:, kt, tc0:tc0 + 128],
                                    wts[j][:, kt, :],
                                    start=(kt == 0), stop=(kt == NKT - 1))
                        for j, vc in enumerate(vcs):
                            ev = lmev.tile([128, 512], F16, tag="lmev",
                                           name="lmev")
                            nc.vector.tensor_copy(ev, pss[j])
                            nc.sync.dma_start(
                                out_d[tt * 128:(tt + 1) * 128,
                                      vc * 512:(vc + 1) * 512], ev)
    nc.finalize()
    return nc


_NC_CACHE = None


def _get_nc():
    global _NC_CACHE
    if _NC_CACHE is None:
        _NC_CACHE = build_bass()
    return _NC_CACHE


def make_in_maps(idx, layer_num, wte, wpe, ln1_g, ln1_b, attn_w, attn_b, proj_w,
                 proj_b, ln2_g, ln2_b, fc_w, fc_b, fc2_w, fc2_b, lnf_g, lnf_b, lm_w):
    bf = ml_dtypes.bfloat16
    idx = np.asarray(idx)
    f32 = np.float32
    wte = np.asarray(wte, f32)
    wpe = np.asarray(wpe, f32)
    x0 = wte[idx] + wpe[:T]                      # [B,T,D] fp32 host embedding
    # layer-0 LN1 on host (saves the first AllGather on device)
    mu = x0.mean(-1, keepdims=True)
    var = x0.var(-1, keepdims=True)
    h0 = ((x0 - mu) / np.sqrt(var + EPS) * np.asarray(ln1_g, f32)[0]
          + np.asarray(ln1_b, f32)[0]).astype(bf)           # [B,T,D]

    qkw = np.ascontiguousarray(np.asarray(attn_w, f32)[:, :, :2 * D]).astype(bf)
    vw = np.ascontiguousarray(np.asarray(attn_w, f32)[:, :, 2 * D:]).astype(bf)
    pw = np.asarray(proj_w, f32).astype(bf)
    fcw = np.asarray(fc_w, f32).astype(bf)
    fc2w = np.asarray(fc2_w, f32).astype(bf)
    qkb = np.ascontiguousarray(np.asarray(attn_b, f32)[:, :2 * D])
    vb = np.ascontiguousarray(np.asarray(attn_b, f32)[:, 2 * D:]).astype(bf)
    lnp = np.stack([np.asarray(ln1_g, f32), np.asarray(ln1_b, f32),
                    np.asarray(ln2_g, f32), np.asarray(ln2_b, f32)],
                   axis=1)                        # [L, 4, D] f32
    lnf = np.stack([np.asarray(lnf_g, f32), np.asarray(lnf_b, f32)], axis=0)

    lmw_pad = np.zeros((D, VPAD), f32)
    lmw_pad[:, :V] = np.asarray(lm_w, f32)
    lmw_bf = lmw_pad.astype(bf)

    in_maps = []
    for core in range(8):
        b = core // 2
        p = core % 2
        own = np.concatenate([np.arange(128) + (2 * s + p) * 128
                              for s in range(NOS)])          # own global tokens
        # per-strip tail mask: cols 0:128 = key tile kt=2s, cols 128:256 =
        # kt=2s+1, vs own q chunk 2s+p (pattern is s-independent)
        diag = (np.arange(128)[:, None] <= np.arange(128)[None, :])
        mask = np.empty((128, 256), np.float32)
        if p == 0:
            mask[:, :128] = diag
            mask[:, 128:] = 0.0
        else:
            mask[:, :128] = 1.0
            mask[:, 128:] = diag
        mask = mask.astype(bf)
        in_maps.append(dict(
            xT=np.ascontiguousarray(x0[b].T[:, own]),
            h0=np.ascontiguousarray(h0[b].T),
            h0own=np.ascontiguousarray(h0[b].T[:, own]),
            qkw=qkw, vw=vw, pw=pw, fcw=fcw, fc2w=fc2w,
            qkb=qkb, vb=vb, pb=np.asarray(proj_b, f32),
            fcb=np.asarray(fc_b, f32), fc2b=np.asarray(fc2_b, f32),
            lnp=lnp, lnf=lnf, mask=mask,
            lmw=lmw_bf,
        ))
    return in_maps


def kernel(**inputs):
    global LAST_RESULT
    in_maps = make_in_maps(**inputs)
    nc = _get_nc()
    res = run_bass_kernel_spmd(nc, in_maps, core_ids=list(range(8)), trace=TRACE)
    LAST_RESULT = res

    logits = np.empty((B, T, V), np.float32)
    for core in range(8):
        b = core // 2
        p = core % 2
        o = res.results[core]["out"].astype(np.float32)      # [TOWN, VPAD]
        for s in range(NOS):
            g = 2 * s + p
            logits[b, g * 128:(g + 1) * 128, :] = o[s * 128:(s + 1) * 128, :V]
    return logits
